# revision 1
# baseline (speedup 1.0000x reference)
"""Trainium2 Bass kernel for nn_ConvSPE (depthwise-conv SPE + per-channel contraction).

Math (reference): per bn=(b,nu) row and channel d:
    pe_k = noise / sqrt(num*d)                       (b*num, d, s+2k)
    pe_q = depthwise_valid_xcorr(pe_k, w)            k=200 taps, same filter per channel
    qhat[b,nu,t] = sum_d pe_q[bn,d,t]      * q[b,d,t]
    khat[b,nu,t] = sum_d pe_k[bn,d,t+k//2] * k[b,d,t]

Kernel strategy (8 NeuronCores, data-parallel over the 128 bn rows; 16 rows/core):
  * Host pre-arranges noise into a time-partition-inner fp16 layout
    xf[bn][p, n, d] = noise[bn, d, 128n+p] so the conv becomes 3 PSUM-accumulated
    TensorE matmuls per output block with fixed Toeplitz weights
    W_s[p, m] = w[p + 128s - m] (shared across all channels/rows).
  * qhat: ScalarE copies conv PSUM -> SBUF fp16, VectorE multiplies by
    host-pre-transposed queries (fp16 2x mode) and reduces over d.
  * khat needs no conv: VectorE multiplies xf by a host-shifted/scaled keys
    layout (shift k//2=100 and 1/sqrt(num*d) baked in); GpSimd reduces over d
    with an fp32 add-tree (engine balancing: DVE is the bottleneck).
"""

import math
import numpy as np

_CACHE = {}


def _ensure_paths():
    try:
        import concourse  # noqa: F401
    except ImportError:
        import sys

        for p in ("/opt/trn_rl_repo", "/root/.axon_site/_ro/trn_rl_repo"):
            if p not in sys.path:
                sys.path.insert(0, p)


N_CORES = 8
B, D, L, K, NUM = 4, 64, 4096, 200, 32
NW = 34  # x windows of 128 loaded per row (covers t+j up to 4351)
NT = 32  # output time blocks of 128
NK = 33  # khat product blocks (u = t + 100 spans [0, 4224))
ROWS = 16  # bn rows per core


def _add_tree(eng, pool, acc_out, src, n_outer, bn, mybir):
    """Reduce src [128, n_outer, 64] over the last axis into acc_out [128, n_outer]
    using TT-adds (fp32 after level 1). Works on engines without X-axis reduce."""
    F32 = mybir.dt.float32
    a = pool.tile([128, n_outer, 32], F32, tag="treeA", name=f"treeA_{bn}")
    b = pool.tile([128, n_outer, 16], F32, tag="treeB", name=f"treeB_{bn}")
    eng.tensor_add(a[:], src[:, :, 0:32], src[:, :, 32:64])
    eng.tensor_add(b[:], a[:, :, 0:16], a[:, :, 16:32])
    eng.tensor_add(a[:, :, 0:8], b[:, :, 0:8], b[:, :, 8:16])
    eng.tensor_add(b[:, :, 0:4], a[:, :, 0:4], a[:, :, 4:8])
    eng.tensor_add(a[:, :, 8:10], b[:, :, 0:2], b[:, :, 2:4])
    eng.tensor_add(acc_out, a[:, :, 8], a[:, :, 9])


def build_module():
    """Build + compile the per-core Bass module (identical SPMD program)."""
    _ensure_paths()
    from contextlib import ExitStack

    import concourse.bacc as bacc
    import concourse.mybir as mybir
    import concourse.tile as tile

    F16 = mybir.dt.float16
    F32 = mybir.dt.float32
    X = mybir.AxisListType.X

    nc = bacc.Bacc(
        "TRN2", target_bir_lowering=False, debug=False, num_devices=N_CORES
    )

    xf_d = nc.dram_tensor("xf", [ROWS, 128, NW, D], F16, kind="ExternalInput").ap()
    wq_d = nc.dram_tensor("wq", [3, 128, 128], F16, kind="ExternalInput").ap()
    qt_d = nc.dram_tensor("qt", [128, NT, D], F16, kind="ExternalInput").ap()
    kf_d = nc.dram_tensor("kf", [128, NK, D], F16, kind="ExternalInput").ap()
    qo_d = nc.dram_tensor("qo", [128, ROWS, NT], F32, kind="ExternalOutput").ap()
    ko_d = nc.dram_tensor("ko", [128, ROWS, NK], F32, kind="ExternalOutput").ap()

    with tile.TileContext(nc) as tc, ExitStack() as ctx:
        wp = ctx.enter_context(tc.tile_pool(name="const", bufs=1))
        xp = ctx.enter_context(tc.tile_pool(name="x", bufs=4))
        pp = ctx.enter_context(tc.tile_pool(name="psum", bufs=4, space="PSUM"))
        cp = ctx.enter_context(tc.tile_pool(name="peq", bufs=3))
        qp = ctx.enter_context(tc.tile_pool(name="prodq", bufs=3))
        kpool = ctx.enter_context(tc.tile_pool(name="prodk", bufs=3))
        tp = ctx.enter_context(tc.tile_pool(name="tree", bufs=3))
        op = ctx.enter_context(tc.tile_pool(name="out", bufs=1))

        wts = []
        for s in range(3):
            t = wp.tile([128, 128], F16, tag=f"w{s}")
            nc.sync.dma_start(t[:], wq_d[s])
            wts.append(t)
        qt_t = wp.tile([128, NT, D], F16, tag="qt")
        nc.sync.dma_start(qt_t[:], qt_d[:])
        kf_t = wp.tile([128, NK, D], F16, tag="kf")
        nc.sync.dma_start(kf_t[:], kf_d[:])

        qacc = op.tile([128, ROWS, NT], F32, tag="qa")
        kacc = op.tile([128, ROWS, NK], F32, tag="ka")

        for bn in range(ROWS):
            xt = xp.tile([128, NW, D], F16, tag="xt", name=f"xt_{bn}")
            nc.sync.dma_start(xt[:], xf_d[bn])

            # ---- khat path: pure elementwise + gpsimd reduce tree
            pk = kpool.tile([128, NK, D], F16, tag="pk", name=f"pk_{bn}")
            nc.vector.tensor_mul(pk[:], xt[:, 0:NK, :], kf_t[:])
            _add_tree(nc.gpsimd, tp, kacc[:, bn, :], pk, NK, bn, mybir)

            # ---- qhat path: conv via 3 Toeplitz matmuls per 8-block group.
            # Two 2-bank PSUM halves per row so ACT/DVE drain half 0 while
            # PE still works on half 1.
            for h in range(2):
                ps = pp.tile([128, NT // 2, D], F32, tag="ps", name=f"ps_{bn}_{h}")
                for s in range(3):
                    for g in range(2 * h, 2 * h + 2):
                        nc.tensor.matmul(
                            ps[:, (g - 2 * h) * 8 : (g - 2 * h + 1) * 8, :],
                            wts[s][:],
                            xt[:, g * 8 + s : g * 8 + s + 8, :],
                            start=(s == 0),
                            stop=(s == 2),
                        )
                peq = cp.tile([128, NT // 2, D], F16, tag="peq", name=f"peq_{bn}_{h}")
                nc.scalar.copy(peq[:], ps[:])
                pq = qp.tile([128, NT // 2, D], F16, tag="pq", name=f"pq_{bn}_{h}")
                nc.vector.tensor_mul(
                    pq[:], peq[:], qt_t[:, h * (NT // 2) : (h + 1) * (NT // 2), :]
                )
                nc.vector.reduce_sum(
                    qacc[:, bn, h * (NT // 2) : (h + 1) * (NT // 2)], pq[:], axis=X
                )

        nc.sync.dma_start(qo_d[:], qacc[:])
        nc.sync.dma_start(ko_d[:], kacc[:])

    nc.compile()
    return nc


def _get_module():
    if "nc" not in _CACHE:
        _CACHE["nc"] = build_module()
    return _CACHE["nc"]


def make_in_maps(queries, keys, noise, conv_weight, num):
    """Host-side shard + re-layout (all cheap numpy ops)."""
    num = int(np.asarray(num))
    queries = np.asarray(queries, dtype=np.float32)
    keys = np.asarray(keys, dtype=np.float32)
    noise = np.asarray(noise, dtype=np.float32)
    w = np.asarray(conv_weight, dtype=np.float32)[0, 0, :]
    scale = 1.0 / math.sqrt(num * D)

    # Toeplitz weights (scale folded in): W_s[p, m] = w[p + 128s - m] * scale
    p = np.arange(128)[:, None]
    m = np.arange(128)[None, :]
    Wq = np.zeros((3, 128, 128), np.float32)
    for s in range(3):
        j = p + 128 * s - m
        mask = (j >= 0) & (j < K)
        Wq[s][mask] = w[j[mask]] * scale
    Wq16 = Wq.astype(np.float16)

    # xf[bn][p, n, d] = noise[bn, d, 128n + p]
    xf = (
        noise[:, :, : NW * 128]
        .reshape(B * NUM, D, NW, 128)
        .transpose(0, 3, 2, 1)
        .astype(np.float16)
    )
    # qt[b][p, tau, d] = queries[b, d, 128 tau + p]
    qt = queries.reshape(B, D, NT, 128).transpose(0, 3, 2, 1).astype(np.float16)
    # kf[b][p, n, d] = keys[b, d, 128n + p - 100] * scale (zero out of range)
    kp = np.zeros((B, D, NK * 128), np.float32)
    kp[:, :, K // 2 : K // 2 + L] = keys * scale
    kf = kp.reshape(B, D, NK, 128).transpose(0, 3, 2, 1).astype(np.float16)

    in_maps = []
    for c in range(N_CORES):
        b = c // 2
        in_maps.append(
            {
                "xf": np.ascontiguousarray(xf[ROWS * c : ROWS * (c + 1)]),
                "wq": Wq16,
                "qt": np.ascontiguousarray(qt[b]),
                "kf": np.ascontiguousarray(kf[b]),
            }
        )
    return in_maps


def assemble_outputs(results):
    qhat = np.empty((B * NUM, L), np.float32)
    khat = np.empty((B * NUM, L), np.float32)
    for c in range(N_CORES):
        qo = results[c]["qo"]  # [128, ROWS, NT]
        ko = results[c]["ko"]  # [128, ROWS, NK]
        qhat[ROWS * c : ROWS * (c + 1)] = qo.transpose(1, 2, 0).reshape(ROWS, L)
        kv = ko.transpose(1, 2, 0).reshape(ROWS, NK * 128)
        khat[ROWS * c : ROWS * (c + 1)] = kv[:, K // 2 : K // 2 + L]
    return (
        qhat.reshape(B, NUM, L),
        khat.reshape(B, NUM, L),
    )


def kernel(queries, keys, noise, conv_weight, num):
    _ensure_paths()
    from concourse import bass_utils

    in_maps = make_in_maps(queries, keys, noise, conv_weight, num)
    nc = _get_module()
    res = bass_utils.run_bass_kernel_spmd(nc, in_maps, core_ids=list(range(N_CORES)))
    return assemble_outputs(res.results)



# revision 25
# speedup vs baseline: 1.3481x; 1.3481x over previous
"""Trainium2 Bass kernel for nn_ConvSPE (depthwise-conv SPE + per-channel contraction).

Math (reference): per bn=(b,nu) row and channel d:
    pe_k = noise / sqrt(num*d)                       (b*num, d, s+2k)
    pe_q = depthwise_valid_xcorr(pe_k, w)            k=200 taps, same filter per channel
    qhat[b,nu,t] = sum_d pe_q[bn,d,t]      * q[b,d,t]
    khat[b,nu,t] = sum_d pe_k[bn,d,t+k//2] * k[b,d,t]

Kernel strategy (8 NeuronCores, data-parallel over the 128 bn rows; 16 rows/core):
  * Host pre-arranges noise into a time-partition-inner fp16 layout
    xt[p, n, d] = noise[bn, d, 128n+p] so the conv becomes 3 PSUM-accumulated
    TensorE matmuls per output half with fixed Toeplitz weights
    W_s[p, m] = w[p + 128s - m] (shared across all channels/rows).
  * Per row: two 2-bank PSUM halves so ACT can drain half 0 while PE still
    accumulates half 1 (keeps the PE p-state ramped).
  * qhat: DVE multiplies drained conv output by host-pre-transposed queries
    (fp16 2x mode); khat: DVE multiplies xt by host-shifted/scaled keys.
  * d-reduction is the scarce resource (TensorReduce has no 2x mode, GpSimd
    adds run at 0.42 efficiency): products are collected into 4-row batch
    tiles and reduced with 6-level fp16 add trees, row-batched to amortize
    per-instruction overhead, split between DVE (2x adds) and GpSimd to
    balance both engines.
"""

import math
import numpy as np

_CACHE = {}


def _ensure_paths():
    try:
        import concourse  # noqa: F401
    except ImportError:
        import sys

        for p in ("/opt/trn_rl_repo", "/root/.axon_site/_ro/trn_rl_repo"):
            if p not in sys.path:
                sys.path.insert(0, p)


N_CORES = 8
B, D, L, K, NUM = 4, 64, 4096, 200, 32
NW = 34  # x windows of 128 loaded per row (covers t+j up to 4351)
NT = 32  # output time blocks of 128
NK = 33  # khat product blocks (u = t + 100 spans [0, 4224))
ROWS = 16  # bn rows per core
RB = 4  # rows per reduce batch

# Which reduce batches (4 rows each) run on GpSimd instead of DVE.
# khat products only need the input DMA (no conv), so Pool's slow trees get
# the early khat batches while DVE keeps the conv-gated qhat trees plus the
# final khat batch (short tail).
QHAT_POOL_BATCHES = frozenset()
KHAT_POOL_BATCHES = frozenset({0, 1, 2})
# Batches where Pool does tree level 1 only and DVE finishes levels 2-6.
QHAT_SPLIT_BATCHES = frozenset()
KHAT_SPLIT_BATCHES = frozenset()


def _add_tree(eng, a, b, acc_out, src):
    """Reduce src [128, rb, n, 64] over the last axis into acc_out
    [128, rb, n] with a 6-level fp16 add tree. a/b are scratch tiles of
    shapes [128, rb, n, 32] / [128, rb, n, 16]."""
    eng.tensor_add(a[:], src[:, :, :, 0:32], src[:, :, :, 32:64])
    eng.tensor_add(b[:], a[:, :, :, 0:16], a[:, :, :, 16:32])
    eng.tensor_add(a[:, :, :, 0:8], b[:, :, :, 0:8], b[:, :, :, 8:16])
    eng.tensor_add(b[:, :, :, 0:4], a[:, :, :, 0:4], a[:, :, :, 4:8])
    eng.tensor_add(a[:, :, :, 8:10], b[:, :, :, 0:2], b[:, :, :, 2:4])
    eng.tensor_add(acc_out, a[:, :, :, 8], a[:, :, :, 9])


def _add_tree_tail(eng, a, b, acc_out):
    """Levels 2-6 given level 1 already in a [128, rb, n, 32]."""
    eng.tensor_add(b[:], a[:, :, :, 0:16], a[:, :, :, 16:32])
    eng.tensor_add(a[:, :, :, 0:8], b[:, :, :, 0:8], b[:, :, :, 8:16])
    eng.tensor_add(b[:, :, :, 0:4], a[:, :, :, 0:4], a[:, :, :, 4:8])
    eng.tensor_add(a[:, :, :, 8:10], b[:, :, :, 0:2], b[:, :, :, 2:4])
    eng.tensor_add(acc_out, a[:, :, :, 8], a[:, :, :, 9])


def build_module():
    """Build + compile the per-core Bass module (identical SPMD program)."""
    _ensure_paths()
    from contextlib import ExitStack

    import concourse.bacc as bacc
    import concourse.mybir as mybir
    import concourse.tile as tile

    F16 = mybir.dt.float16
    F32 = mybir.dt.float32

    nc = bacc.Bacc(
        "TRN2", target_bir_lowering=False, debug=False, num_devices=N_CORES
    )

    xf_d = nc.dram_tensor("xf", [ROWS, 128, NW, D], F16, kind="ExternalInput").ap()
    wq_d = nc.dram_tensor("wq", [3, 128, 128], F16, kind="ExternalInput").ap()
    qt_d = nc.dram_tensor("qt", [128, NT, D], F16, kind="ExternalInput").ap()
    kf_d = nc.dram_tensor("kf", [128, NK, D], F16, kind="ExternalInput").ap()
    qo_d = nc.dram_tensor("qo", [128, ROWS, NT], F32, kind="ExternalOutput").ap()
    ko_d = nc.dram_tensor("ko", [128, ROWS, NK], F32, kind="ExternalOutput").ap()

    NB = ROWS // RB

    with tile.TileContext(nc) as tc, ExitStack() as ctx:
        wp = ctx.enter_context(tc.tile_pool(name="const", bufs=1))
        xp = ctx.enter_context(tc.tile_pool(name="x", bufs=4))
        pp = ctx.enter_context(tc.tile_pool(name="psum", bufs=4, space="PSUM"))
        cp = ctx.enter_context(tc.tile_pool(name="peq", bufs=3))
        qp = ctx.enter_context(tc.tile_pool(name="prodq", bufs=3))
        kpool = ctx.enter_context(tc.tile_pool(name="prodk", bufs=3))
        tp = ctx.enter_context(tc.tile_pool(name="tree", bufs=2))
        op = ctx.enter_context(tc.tile_pool(name="out", bufs=1))

        wts = []
        for s in range(3):
            t = wp.tile([128, 128], F16, tag=f"w{s}")
            nc.sync.dma_start(t[:], wq_d[s])
            wts.append(t)
        xts = []
        for bn in range(2):
            xt = xp.tile([128, NW, D], F16, tag="xt", name=f"xt_{bn}")
            nc.sync.dma_start(xt[:], xf_d[bn])
            xts.append(xt)
        kf_t = wp.tile([128, NK, D], F16, tag="kf")
        nc.sync.dma_start(kf_t[:], kf_d[:])
        qt_t = wp.tile([128, NT, D], F16, tag="qt")
        nc.sync.dma_start(qt_t[:], qt_d[:])

        qacc = op.tile([128, ROWS, NT], F32, tag="qa")
        kacc = op.tile([128, ROWS, NK], F32, tag="ka")

        # deferred DVE tree jobs: emitted one batch late so DVE's muls keep
        # feeding Pool's trees without serializing behind DVE's own trees
        pending_prev = []
        pending_cur = []

        for rb in range(NB):
            pqb = qp.tile([128, RB, NT, D], F16, tag="pq", name=f"pq_{rb}")
            pkb = kpool.tile([128, RB, NK, D], F16, tag="pk", name=f"pk_{rb}")

            # khat products first: they gate Pool's trees and need no conv
            for r in range(RB):
                bn = rb * RB + r
                if bn < 2:
                    xt = xts[bn]
                else:
                    xt = xp.tile([128, NW, D], F16, tag="xt", name=f"xt_{bn}")
                    nc.sync.dma_start(xt[:], xf_d[bn])
                    xts.append(xt)
                nc.vector.tensor_mul(pkb[:, r], xts[bn][:, 0:NK, :], kf_t[:])

            # ---- batched khat reduce tree over d (64 -> 1)
            ka = tp.tile([128, RB, NK, 32], F16, tag="ka", name=f"ktA_{rb}")
            kb = tp.tile([128, RB, NK, 16], F16, tag="kb", name=f"ktB_{rb}")
            kout = kacc[:, rb * RB : (rb + 1) * RB, :]
            if rb in KHAT_POOL_BATCHES:
                _add_tree(nc.gpsimd, ka, kb, kout, pkb)
            else:
                pending_cur.append(
                    lambda ka=ka, kb=kb, kout=kout, pkb=pkb: _add_tree(
                        nc.vector, ka, kb, kout, pkb
                    )
                )

            for r in range(RB):
                bn = rb * RB + r
                xt = xts[bn]
                # conv: 3 Toeplitz matmuls per 8-block group, 2 PSUM halves
                # per row so ACT drains half 0 while PE accumulates half 1
                for h in range(2):
                    ps = pp.tile(
                        [128, NT // 2, D], F32, tag="ps", name=f"ps_{bn}_{h}"
                    )
                    for s in range(3):
                        for g in range(2 * h, 2 * h + 2):
                            nc.tensor.matmul(
                                ps[:, (g - 2 * h) * 8 : (g - 2 * h + 1) * 8, :],
                                wts[s][:],
                                xt[:, g * 8 + s : g * 8 + s + 8, :],
                                start=(s == 0),
                                stop=(s == 2),
                            )
                    peq = cp.tile(
                        [128, NT // 2, D], F16, tag="peq", name=f"peq_{bn}_{h}"
                    )
                    nc.scalar.copy(peq[:], ps[:])
                    nc.vector.tensor_mul(
                        pqb[:, r, h * (NT // 2) : (h + 1) * (NT // 2), :],
                        peq[:],
                        qt_t[:, h * (NT // 2) : (h + 1) * (NT // 2), :],
                    )

            # ---- batched qhat reduce tree; last batch split in two for a
            # shorter tail (second half only waits on rows 14-15)
            halves = (
                [(0, RB)]
                if rb < NB - 1
                else [(0, RB // 2), (RB // 2, RB)]
            )
            for h0, h1 in halves:
                qa = tp.tile(
                    [128, h1 - h0, NT, 32], F16, tag="qa", name=f"qtA_{rb}_{h0}"
                )
                qb = tp.tile(
                    [128, h1 - h0, NT, 16], F16, tag="qb", name=f"qtB_{rb}_{h0}"
                )
                qout = qacc[:, rb * RB + h0 : rb * RB + h1, :]
                pqs = pqb[:, h0:h1]
                if rb in QHAT_POOL_BATCHES:
                    _add_tree(nc.gpsimd, qa, qb, qout, pqs)
                else:
                    pending_cur.append(
                        lambda qa=qa, qb=qb, qout=qout, pqs=pqs: _add_tree(
                            nc.vector, qa, qb, qout, pqs
                        )
                    )

            # emit the previous batch's DVE trees behind this batch's muls
            for job in pending_prev:
                job()
            pending_prev = pending_cur
            pending_cur = []

        for job in pending_prev:
            job()

        nc.sync.dma_start(qo_d[:], qacc[:])
        nc.sync.dma_start(ko_d[:], kacc[:])

    nc.compile()
    return nc


def _get_module():
    if "nc" not in _CACHE:
        _CACHE["nc"] = build_module()
    return _CACHE["nc"]


def make_in_maps(queries, keys, noise, conv_weight, num):
    """Host-side shard + re-layout (all cheap numpy ops)."""
    num = int(np.asarray(num))
    queries = np.asarray(queries, dtype=np.float32)
    keys = np.asarray(keys, dtype=np.float32)
    noise = np.asarray(noise, dtype=np.float32)
    w = np.asarray(conv_weight, dtype=np.float32)[0, 0, :]
    scale = 1.0 / math.sqrt(num * D)

    # Toeplitz weights (scale folded in): W_s[p, m] = w[p + 128s - m] * scale
    p = np.arange(128)[:, None]
    m = np.arange(128)[None, :]
    Wq = np.zeros((3, 128, 128), np.float32)
    for s in range(3):
        j = p + 128 * s - m
        mask = (j >= 0) & (j < K)
        Wq[s][mask] = w[j[mask]] * scale
    Wq16 = Wq.astype(np.float16)

    # xf[bn][p, n, d] = noise[bn, d, 128n + p]
    xf = (
        noise[:, :, : NW * 128]
        .reshape(B * NUM, D, NW, 128)
        .transpose(0, 3, 2, 1)
        .astype(np.float16)
    )
    # qt[b][p, tau, d] = queries[b, d, 128 tau + p]
    qt = queries.reshape(B, D, NT, 128).transpose(0, 3, 2, 1).astype(np.float16)
    # kf[b][p, n, d] = keys[b, d, 128n + p - 100] * scale (zero out of range)
    kp = np.zeros((B, D, NK * 128), np.float32)
    kp[:, :, K // 2 : K // 2 + L] = keys * scale
    kf = kp.reshape(B, D, NK, 128).transpose(0, 3, 2, 1).astype(np.float16)

    in_maps = []
    for c in range(N_CORES):
        b = c // 2
        in_maps.append(
            {
                "xf": np.ascontiguousarray(xf[ROWS * c : ROWS * (c + 1)]),
                "wq": Wq16,
                "qt": np.ascontiguousarray(qt[b]),
                "kf": np.ascontiguousarray(kf[b]),
            }
        )
    return in_maps


def assemble_outputs(results):
    qhat = np.empty((B * NUM, L), np.float32)
    khat = np.empty((B * NUM, L), np.float32)
    for c in range(N_CORES):
        qo = results[c]["qo"]  # [128, ROWS, NT]
        ko = results[c]["ko"]  # [128, ROWS, NK]
        qhat[ROWS * c : ROWS * (c + 1)] = qo.transpose(1, 2, 0).reshape(ROWS, L)
        kv = ko.transpose(1, 2, 0).reshape(ROWS, NK * 128)
        khat[ROWS * c : ROWS * (c + 1)] = kv[:, K // 2 : K // 2 + L]
    return (
        qhat.reshape(B, NUM, L),
        khat.reshape(B, NUM, L),
    )


def kernel(queries, keys, noise, conv_weight, num):
    _ensure_paths()
    from concourse import bass_utils

    in_maps = make_in_maps(queries, keys, noise, conv_weight, num)
    nc = _get_module()
    res = bass_utils.run_bass_kernel_spmd(nc, in_maps, core_ids=list(range(N_CORES)))
    return assemble_outputs(res.results)


# revision 26
# speedup vs baseline: 1.3483x; 1.0001x over previous
"""Trainium2 Bass kernel for nn_ConvSPE (depthwise-conv SPE + per-channel contraction).

Math (reference): per bn=(b,nu) row and channel d:
    pe_k = noise / sqrt(num*d)                       (b*num, d, s+2k)
    pe_q = depthwise_valid_xcorr(pe_k, w)            k=200 taps, same filter per channel
    qhat[b,nu,t] = sum_d pe_q[bn,d,t]      * q[b,d,t]
    khat[b,nu,t] = sum_d pe_k[bn,d,t+k//2] * k[b,d,t]

Kernel strategy (8 NeuronCores, data-parallel over the 128 bn rows; 16 rows/core):
  * Host pre-arranges noise into a time-partition-inner fp16 layout
    xt[p, n, d] = noise[bn, d, 128n+p] so the conv becomes 3 PSUM-accumulated
    TensorE matmuls per output half with fixed Toeplitz weights
    W_s[p, m] = w[p + 128s - m] (shared across all channels/rows).
  * Per row: two 2-bank PSUM halves so ACT can drain half 0 while PE still
    accumulates half 1 (keeps the PE p-state ramped).
  * qhat: DVE multiplies drained conv output by host-pre-transposed queries
    (fp16 2x mode); khat: DVE multiplies xt by host-shifted/scaled keys.
  * d-reduction is the scarce resource (TensorReduce has no 2x mode, GpSimd
    adds run at 0.42 efficiency): products are collected into 4-row batch
    tiles and reduced with 6-level fp16 add trees, row-batched to amortize
    per-instruction overhead, split between DVE (2x adds) and GpSimd to
    balance both engines.
"""

import math
import numpy as np

_CACHE = {}


def _ensure_paths():
    try:
        import concourse  # noqa: F401
    except ImportError:
        import sys

        for p in ("/opt/trn_rl_repo", "/root/.axon_site/_ro/trn_rl_repo"):
            if p not in sys.path:
                sys.path.insert(0, p)


N_CORES = 8
B, D, L, K, NUM = 4, 64, 4096, 200, 32
NW = 34  # x windows of 128 loaded per row (covers t+j up to 4351)
NT = 32  # output time blocks of 128
NK = 33  # khat product blocks (u = t + 100 spans [0, 4224))
ROWS = 16  # bn rows per core
RB = 4  # rows per reduce batch

# Which reduce batches (4 rows each) run on GpSimd instead of DVE.
# khat products only need the input DMA (no conv), so Pool's slow trees get
# the early khat batches while DVE keeps the conv-gated qhat trees plus the
# final khat batch (short tail).
QHAT_POOL_BATCHES = frozenset()
KHAT_POOL_BATCHES = frozenset({0, 1, 2})
# Batches where Pool does tree level 1 only and DVE finishes levels 2-6.
QHAT_SPLIT_BATCHES = frozenset()
KHAT_SPLIT_BATCHES = frozenset()


def _add_tree(eng, a, b, acc_out, src):
    """Reduce src [128, rb, n, 64] over the last axis into acc_out
    [128, rb, n] with a 6-level fp16 add tree. a/b are scratch tiles of
    shapes [128, rb, n, 32] / [128, rb, n, 16]."""
    eng.tensor_add(a[:], src[:, :, :, 0:32], src[:, :, :, 32:64])
    eng.tensor_add(b[:], a[:, :, :, 0:16], a[:, :, :, 16:32])
    eng.tensor_add(a[:, :, :, 0:8], b[:, :, :, 0:8], b[:, :, :, 8:16])
    eng.tensor_add(b[:, :, :, 0:4], a[:, :, :, 0:4], a[:, :, :, 4:8])
    eng.tensor_add(a[:, :, :, 8:10], b[:, :, :, 0:2], b[:, :, :, 2:4])
    eng.tensor_add(acc_out, a[:, :, :, 8], a[:, :, :, 9])


def _add_tree_tail(eng, a, b, acc_out):
    """Levels 2-6 given level 1 already in a [128, rb, n, 32]."""
    eng.tensor_add(b[:], a[:, :, :, 0:16], a[:, :, :, 16:32])
    eng.tensor_add(a[:, :, :, 0:8], b[:, :, :, 0:8], b[:, :, :, 8:16])
    eng.tensor_add(b[:, :, :, 0:4], a[:, :, :, 0:4], a[:, :, :, 4:8])
    eng.tensor_add(a[:, :, :, 8:10], b[:, :, :, 0:2], b[:, :, :, 2:4])
    eng.tensor_add(acc_out, a[:, :, :, 8], a[:, :, :, 9])


def build_module():
    """Build + compile the per-core Bass module (identical SPMD program)."""
    _ensure_paths()
    from contextlib import ExitStack

    import concourse.bacc as bacc
    import concourse.mybir as mybir
    import concourse.tile as tile

    F16 = mybir.dt.float16
    F32 = mybir.dt.float32

    nc = bacc.Bacc(
        "TRN2", target_bir_lowering=False, debug=False, num_devices=N_CORES
    )

    xf_d = nc.dram_tensor("xf", [ROWS, 128, NW, D], F16, kind="ExternalInput").ap()
    wq_d = nc.dram_tensor("wq", [3, 128, 128], F16, kind="ExternalInput").ap()
    qt_d = nc.dram_tensor("qt", [128, NT, D], F16, kind="ExternalInput").ap()
    kf_d = nc.dram_tensor("kf", [128, NK, D], F16, kind="ExternalInput").ap()
    qo_d = nc.dram_tensor("qo", [128, ROWS, NT], F32, kind="ExternalOutput").ap()
    ko_d = nc.dram_tensor("ko", [128, ROWS, NK], F32, kind="ExternalOutput").ap()

    NB = ROWS // RB

    with tile.TileContext(nc) as tc, ExitStack() as ctx:
        wp = ctx.enter_context(tc.tile_pool(name="const", bufs=1))
        xp = ctx.enter_context(tc.tile_pool(name="x", bufs=4))
        pp = ctx.enter_context(tc.tile_pool(name="psum", bufs=4, space="PSUM"))
        cp = ctx.enter_context(tc.tile_pool(name="peq", bufs=3))
        qp = ctx.enter_context(tc.tile_pool(name="prodq", bufs=3))
        kpool = ctx.enter_context(tc.tile_pool(name="prodk", bufs=3))
        tp = ctx.enter_context(tc.tile_pool(name="tree", bufs=2))
        op = ctx.enter_context(tc.tile_pool(name="out", bufs=1))

        wts = []
        for s in range(3):
            t = wp.tile([128, 128], F16, tag=f"w{s}")
            nc.sync.dma_start(t[:], wq_d[s])
            wts.append(t)
        xts = []
        xt = xp.tile([128, NW, D], F16, tag="xt", name="xt_0")
        nc.sync.dma_start(xt[:], xf_d[0])
        xts.append(xt)
        kf_t = wp.tile([128, NK, D], F16, tag="kf")
        nc.sync.dma_start(kf_t[:], kf_d[:])
        xt = xp.tile([128, NW, D], F16, tag="xt", name="xt_1")
        nc.sync.dma_start(xt[:], xf_d[1])
        xts.append(xt)
        qt_t = wp.tile([128, NT, D], F16, tag="qt")
        nc.sync.dma_start(qt_t[:], qt_d[:])

        qacc = op.tile([128, ROWS, NT], F32, tag="qa")
        kacc = op.tile([128, ROWS, NK], F32, tag="ka")

        # deferred DVE tree jobs: emitted one batch late so DVE's muls keep
        # feeding Pool's trees without serializing behind DVE's own trees
        pending_prev = []
        pending_cur = []

        for rb in range(NB):
            pqb = qp.tile([128, RB, NT, D], F16, tag="pq", name=f"pq_{rb}")
            pkb = kpool.tile([128, RB, NK, D], F16, tag="pk", name=f"pk_{rb}")

            # khat products first: they gate Pool's trees and need no conv
            for r in range(RB):
                bn = rb * RB + r
                if bn < 2:
                    xt = xts[bn]
                else:
                    xt = xp.tile([128, NW, D], F16, tag="xt", name=f"xt_{bn}")
                    nc.sync.dma_start(xt[:], xf_d[bn])
                    xts.append(xt)
                nc.vector.tensor_mul(pkb[:, r], xts[bn][:, 0:NK, :], kf_t[:])

            # ---- batched khat reduce tree over d (64 -> 1)
            ka = tp.tile([128, RB, NK, 32], F16, tag="ka", name=f"ktA_{rb}")
            kb = tp.tile([128, RB, NK, 16], F16, tag="kb", name=f"ktB_{rb}")
            kout = kacc[:, rb * RB : (rb + 1) * RB, :]
            if rb in KHAT_POOL_BATCHES:
                _add_tree(nc.gpsimd, ka, kb, kout, pkb)
            else:
                pending_cur.append(
                    lambda ka=ka, kb=kb, kout=kout, pkb=pkb: _add_tree(
                        nc.vector, ka, kb, kout, pkb
                    )
                )

            for r in range(RB):
                bn = rb * RB + r
                xt = xts[bn]
                # conv: 3 Toeplitz matmuls per 8-block group, 2 PSUM halves
                # per row so ACT drains half 0 while PE accumulates half 1
                for h in range(2):
                    ps = pp.tile(
                        [128, NT // 2, D], F32, tag="ps", name=f"ps_{bn}_{h}"
                    )
                    for s in range(3):
                        for g in range(2 * h, 2 * h + 2):
                            nc.tensor.matmul(
                                ps[:, (g - 2 * h) * 8 : (g - 2 * h + 1) * 8, :],
                                wts[s][:],
                                xt[:, g * 8 + s : g * 8 + s + 8, :],
                                start=(s == 0),
                                stop=(s == 2),
                            )
                    peq = cp.tile(
                        [128, NT // 2, D], F16, tag="peq", name=f"peq_{bn}_{h}"
                    )
                    nc.scalar.copy(peq[:], ps[:])
                    nc.vector.tensor_mul(
                        pqb[:, r, h * (NT // 2) : (h + 1) * (NT // 2), :],
                        peq[:],
                        qt_t[:, h * (NT // 2) : (h + 1) * (NT // 2), :],
                    )

            # ---- batched qhat reduce tree; last batch split in two for a
            # shorter tail (second half only waits on rows 14-15)
            halves = (
                [(0, RB)]
                if rb < NB - 1
                else [(0, RB // 2), (RB // 2, RB)]
            )
            for h0, h1 in halves:
                qa = tp.tile(
                    [128, h1 - h0, NT, 32], F16, tag="qa", name=f"qtA_{rb}_{h0}"
                )
                qb = tp.tile(
                    [128, h1 - h0, NT, 16], F16, tag="qb", name=f"qtB_{rb}_{h0}"
                )
                qout = qacc[:, rb * RB + h0 : rb * RB + h1, :]
                pqs = pqb[:, h0:h1]
                if rb in QHAT_POOL_BATCHES:
                    _add_tree(nc.gpsimd, qa, qb, qout, pqs)
                else:
                    pending_cur.append(
                        lambda qa=qa, qb=qb, qout=qout, pqs=pqs: _add_tree(
                            nc.vector, qa, qb, qout, pqs
                        )
                    )

            # emit the previous batch's DVE trees behind this batch's muls
            for job in pending_prev:
                job()
            pending_prev = pending_cur
            pending_cur = []

        for job in pending_prev:
            job()

        nc.sync.dma_start(qo_d[:], qacc[:])
        nc.sync.dma_start(ko_d[:], kacc[:])

    nc.compile()
    return nc


def _get_module():
    if "nc" not in _CACHE:
        _CACHE["nc"] = build_module()
    return _CACHE["nc"]


def make_in_maps(queries, keys, noise, conv_weight, num):
    """Host-side shard + re-layout (all cheap numpy ops)."""
    num = int(np.asarray(num))
    queries = np.asarray(queries, dtype=np.float32)
    keys = np.asarray(keys, dtype=np.float32)
    noise = np.asarray(noise, dtype=np.float32)
    w = np.asarray(conv_weight, dtype=np.float32)[0, 0, :]
    scale = 1.0 / math.sqrt(num * D)

    # Toeplitz weights (scale folded in): W_s[p, m] = w[p + 128s - m] * scale
    p = np.arange(128)[:, None]
    m = np.arange(128)[None, :]
    Wq = np.zeros((3, 128, 128), np.float32)
    for s in range(3):
        j = p + 128 * s - m
        mask = (j >= 0) & (j < K)
        Wq[s][mask] = w[j[mask]] * scale
    Wq16 = Wq.astype(np.float16)

    # xf[bn][p, n, d] = noise[bn, d, 128n + p]
    xf = (
        noise[:, :, : NW * 128]
        .reshape(B * NUM, D, NW, 128)
        .transpose(0, 3, 2, 1)
        .astype(np.float16)
    )
    # qt[b][p, tau, d] = queries[b, d, 128 tau + p]
    qt = queries.reshape(B, D, NT, 128).transpose(0, 3, 2, 1).astype(np.float16)
    # kf[b][p, n, d] = keys[b, d, 128n + p - 100] * scale (zero out of range)
    kp = np.zeros((B, D, NK * 128), np.float32)
    kp[:, :, K // 2 : K // 2 + L] = keys * scale
    kf = kp.reshape(B, D, NK, 128).transpose(0, 3, 2, 1).astype(np.float16)

    in_maps = []
    for c in range(N_CORES):
        b = c // 2
        in_maps.append(
            {
                "xf": np.ascontiguousarray(xf[ROWS * c : ROWS * (c + 1)]),
                "wq": Wq16,
                "qt": np.ascontiguousarray(qt[b]),
                "kf": np.ascontiguousarray(kf[b]),
            }
        )
    return in_maps


def assemble_outputs(results):
    qhat = np.empty((B * NUM, L), np.float32)
    khat = np.empty((B * NUM, L), np.float32)
    for c in range(N_CORES):
        qo = results[c]["qo"]  # [128, ROWS, NT]
        ko = results[c]["ko"]  # [128, ROWS, NK]
        qhat[ROWS * c : ROWS * (c + 1)] = qo.transpose(1, 2, 0).reshape(ROWS, L)
        kv = ko.transpose(1, 2, 0).reshape(ROWS, NK * 128)
        khat[ROWS * c : ROWS * (c + 1)] = kv[:, K // 2 : K // 2 + L]
    return (
        qhat.reshape(B, NUM, L),
        khat.reshape(B, NUM, L),
    )


def kernel(queries, keys, noise, conv_weight, num):
    _ensure_paths()
    from concourse import bass_utils

    in_maps = make_in_maps(queries, keys, noise, conv_weight, num)
    nc = _get_module()
    res = bass_utils.run_bass_kernel_spmd(nc, in_maps, core_ids=list(range(N_CORES)))
    return assemble_outputs(res.results)
